# revision 41
# baseline (speedup 1.0000x reference)
"""Trainium2 Bass kernel for nn_AttrAttentionLayer (GAT-style attention layer).

Reference computation per batch element b (N=2048 nodes, F_in=256, F_out=64):
    Wh = h @ W                                  [N, F_out]
    f1 = Wh @ a1 ; f2 = Wh @ a2                 [N]
    e  = leaky_relu(f1[:,None] + f2[None,:], 0.2) * node_type
    att= softmax(where(adj>0, e, -9e15), axis=0)   (softmax over i, per column j)
    out= relu(att @ (Wh * level[:,None]))       [N, F_out]

Sharding: batch dim B=8 -> one batch element per NeuronCore (pure data
parallel, no collectives).

Per-core algorithm (all scores kept on-chip, adj/node_type streamed once):
  - Wh/f1/f2 via PE matmuls from on-chip-transposed h tiles.
  - Scores computed in natural [i_part, j_free] layout:
        s = Lrelu(F2bc + f1_i)      (ScalarE, per-partition bias)
        s = s * node_type           (VectorE)
        s = (s + 60) * float(adj)   (VectorE scalar_tensor_tensor; adj
                                     converted int32->f32 on GpSimd)
    so masked entries become exactly 0 and unmasked (s+60).
  - PE transposes 128x128 score blocks into PSUM; ScalarE evicts with
    Exp(x - 60) into a resident bf16 P^T [j_part, i_free] (8MB), also
    emitting per-column sums via accum_out.  exp(0-60)~8.8e-27 reproduces
    the reference's exp(-9e15 - max) = 0 for masked entries.
  - whl[j,o] = Wh[j,o] * level[j] / colsum[j]  (bf16)
  - h'^T[o,i] = sum_j whl[j,o] * P^T[j,i]  (PSUM-accumulated matmuls),
    Relu on evict; final PE transpose back to [i, o].
"""

import os
import sys

import numpy as np

_REPO = "/opt/trn_rl_repo"
if _REPO not in sys.path:
    sys.path.insert(0, _REPO)

import concourse.bass as bass  # noqa: E402
import concourse.tile as tile  # noqa: E402
from concourse import bacc, masks, mybir  # noqa: E402

FP32 = mybir.dt.float32
BF16 = mybir.dt.bfloat16
I32 = mybir.dt.int32

ALPHA = 0.2
MASK_SHIFT = 60.0


class Cfg:
    def __init__(self, N=2048, F_in=256, F_out=64, lrelu_engine="act",
                 mixed_int=True, sc_bufs=6):
        assert N % 128 == 0 and F_in % 128 == 0
        self.N, self.F_in, self.F_out = N, F_in, F_out
        self.NTI = N // 128            # i/j tiles of 128 rows
        self.GRP = min(4, self.NTI)    # i-tiles per transpose group
        assert self.NTI % self.GRP == 0
        self.NIG = self.NTI // self.GRP
        self.IC = 128 * self.GRP       # i-chunk (psum free dim), <= 512
        self.NFC = F_in // 128         # f-blocks of contraction dim
        self.lrelu_engine = lrelu_engine   # "act" (HW only), "dve", "gpsimd"
        self.mixed_int = mixed_int     # int32 operand in DVE tensor ops
        self.sc_bufs = sc_bufs


def attn_kernel(tc: tile.TileContext, out_ap, in_aps, cfg: Cfg):
    """Emit the per-core kernel. in_aps: dict name -> bass.AP."""
    from contextlib import ExitStack

    nc = tc.nc
    N, F_in, F_out = cfg.N, cfg.F_in, cfg.F_out
    NTI, GRP, NIG, IC, NFC = cfg.NTI, cfg.GRP, cfg.NIG, cfg.IC, cfg.NFC

    h_d = in_aps["h"]
    adj_d = in_aps["adj"]
    nt_d = in_aps["node_type"]
    level_d = in_aps["level"]
    W_d = in_aps["W"]
    a_d = in_aps["a"]

    with ExitStack() as ctx:
        # ---------- persistent SBUF ----------
        persist = ctx.enter_context(tc.tile_pool(name="persist", bufs=1))
        id128 = persist.tile([128, 128], FP32, tag="id128")
        masks.make_identity(nc, id128[:])

        pt_all = persist.tile([128, NTI, N], BF16, tag="pt")      # P^T tiles
        f2bc = persist.tile([128, N], FP32, tag="f2bc")           # f2 bcast
        f1_all = persist.tile([128, NTI], FP32, tag="f1")         # f1 cols
        wh_all = persist.tile([128, NTI * F_out], FP32, tag="wh")
        whl2 = persist.tile([128, NTI * F_out], BF16, tag="whl2")
        cs_parts = persist.tile([128, NTI * NIG], FP32, tag="csp")
        cs = persist.tile([128, NTI], FP32, tag="cs")
        inv_cs = persist.tile([128, NTI], FP32, tag="invcs")
        level_sb = persist.tile([128, NTI], FP32, tag="level")
        hpT = persist.tile([F_out, N], FP32, tag="hpT")           # h'^T
        W_sb = persist.tile([128, NFC, F_out], FP32, tag="W")
        ones_sb = persist.tile([1, 128], FP32, tag="ones")
        nc.vector.memset(ones_sb[:], 1.0)
        neg_shift = persist.tile([128, 1], FP32, tag="negshift")
        nc.vector.memset(neg_shift[:], -500.0)
        # 500*I in bf16: adds 500*adj^T onto transposed scores via PE
        id500 = persist.tile([128, 128], BF16, tag="id500")
        nc.gpsimd.memset(id500[:], 0.0)
        nc.gpsimd.affine_select(
            out=id500[:], in_=id500[:],
            compare_op=mybir.AluOpType.not_equal, fill=500.0,
            base=0, pattern=[[-1, 128]], channel_multiplier=1)

        # level[j]: tile tj's rows as column tj -> rearrange "(t p) -> p t"
        nc.sync.dma_start(out=level_sb[:, :],
                          in_=level_d.rearrange("(t p) -> p t", p=128))
        for c in range(NFC):
            nc.sync.dma_start(out=W_sb[:, c, :],
                              in_=W_d[c * 128:(c + 1) * 128, :])

        # ---------- phase 1: f1/f2 fast path + Wh ----------
        # wa = W @ a (on-device, via W-transposes), then
        # f1[i] = sum_f h[i,f]*wa1[f] as the accum_out of (h * wa1bc) on
        # DVE -- depends only on the h tile, so f2bc (the gate for all
        # score work) is ready ~15us in, while Wh matmuls run alongside.
        with ExitStack() as p1:
            sb1 = p1.enter_context(tc.tile_pool(name="sb1", bufs=3))
            psA = p1.enter_context(tc.tile_pool(name="psA", bufs=2, space="PSUM"))
            psB = p1.enter_context(tc.tile_pool(name="psB", bufs=1, space="PSUM"))

            # WT [F_out, F_in] from W tiles; wa_rows = [a1 a2]^T @ WT^T
            wt_ps = psB.tile([F_out, F_in], FP32, tag="misc")
            for c in range(NFC):
                nc.tensor.transpose(wt_ps[:, c * 128:(c + 1) * 128],
                                    W_sb[:, c, :], id128[:])
            wt_sb = sb1.tile([F_out, F_in], FP32, tag="wtsb")
            nc.vector.tensor_copy(wt_sb[:], wt_ps[:])
            a2x = sb1.tile([F_out, 2], FP32, tag="a2x")
            nc.sync.dma_start(out=a2x[:],
                              in_=a_d.rearrange("(c o) one -> o (c one)", c=2))
            # wa_k rows (one matmul per k: lhsT must sit at partition 0)
            wa_rows = sb1.tile([1, 2, F_in], FP32, tag="warows")
            for k in range(2):
                wa_ps = psB.tile([1, F_in], FP32, tag="misc")
                nc.tensor.matmul(wa_ps[:], a2x[:, k:k + 1], wt_sb[:],
                                 start=True, stop=True)
                nc.vector.tensor_copy(wa_rows[:, k, :], wa_ps[:])
            # broadcast wa1/wa2 across all 128 partitions
            wabc = persist.tile([128, 2, F_in], FP32, tag="wabc")
            for k in range(2):
                wab_ps = psB.tile([128, F_in], FP32, tag="misc")
                nc.tensor.matmul(wab_ps[:], ones_sb[:], wa_rows[:, k, :],
                                 start=True, stop=True)
                nc.vector.tensor_copy(wabc[:, k, :], wab_ps[:])

            f2col = persist.tile([128, NTI], FP32, tag="f2col")
            fscr = sb1.tile([128, F_in], FP32, tag="fscr")
            for ti in range(NTI):
                h_t = sb1.tile([128, F_in], FP32, tag="h")
                nc.sync.dma_start(out=h_t[:], in_=h_d[ti * 128:(ti + 1) * 128, :])
                nc.vector.scalar_tensor_tensor(
                    out=fscr[:], in0=h_t[:], scalar=1.0, in1=wabc[:, 0, :],
                    op0=mybir.AluOpType.mult, op1=mybir.AluOpType.mult,
                    accum_out=f1_all[:, ti:ti + 1])
                nc.vector.scalar_tensor_tensor(
                    out=fscr[:], in0=h_t[:], scalar=1.0, in1=wabc[:, 1, :],
                    op0=mybir.AluOpType.mult, op1=mybir.AluOpType.mult,
                    accum_out=f2col[:, ti:ti + 1])

                hT_ps = psA.tile([128, F_in], FP32, tag="hT")
                for c in range(NFC):
                    nc.tensor.transpose(hT_ps[:, c * 128:(c + 1) * 128],
                                        h_t[:, c * 128:(c + 1) * 128], id128[:])
                hT_sb = sb1.tile([128, F_in], FP32, tag="hTsb")
                nc.vector.tensor_copy(hT_sb[:], hT_ps[:])

                wh_ps = psA.tile([128, F_out], FP32, tag="whps")
                for c in range(NFC):
                    nc.tensor.matmul(wh_ps[:], hT_sb[:, c * 128:(c + 1) * 128],
                                     W_sb[:, c, :],
                                     start=(c == 0), stop=(c == NFC - 1))
                nc.scalar.copy(wh_all[:, ti * F_out:(ti + 1) * F_out],
                               wh_ps[:])

            # ---------- phase 2: f2col -> [1, N] row -> broadcast ---------
            f2t_ps = psB.tile([NTI, 128], FP32, tag="misc")
            nc.tensor.transpose(f2t_ps[:], f2col[:], id128[:])
            f2seg = sb1.tile([NTI, 128], FP32, tag="f2seg")
            nc.vector.tensor_copy(f2seg[:], f2t_ps[:])
            f2row = sb1.tile([1, N], FP32, tag="f2row")
            # flatten partitions of f2seg into partition-0 free space
            nc.sync.dma_start(
                out=f2row[0:1, :].rearrange("one (a b) -> one a b", a=NTI),
                in_=f2seg[:, :])
            for c0 in range(0, N, 512):
                w = min(512, N - c0)
                bc_ps = psB.tile([128, 512], FP32, tag="misc")
                nc.tensor.matmul(bc_ps[:, :w], ones_sb[:],
                                 f2row[:, c0:c0 + w], start=True, stop=True)
                nc.vector.tensor_copy(f2bc[:, c0:c0 + w], bc_ps[:, :w])

        # ---------- phases 3-5 ----------
        with ExitStack() as p3:
            ioa = p3.enter_context(tc.tile_pool(name="ioa", bufs=2))
            io = p3.enter_context(tc.tile_pool(name="io", bufs=3))
            cvt = p3.enter_context(tc.tile_pool(name="cvt", bufs=GRP + 1))
            sc = p3.enter_context(tc.tile_pool(name="sc", bufs=cfg.sc_bufs))
            ps_tp = p3.enter_context(tc.tile_pool(name="pstp", bufs=3, space="PSUM"))
            ps_mm = p3.enter_context(tc.tile_pool(name="psmm", bufs=2, space="PSUM"))
            out_pool = p3.enter_context(tc.tile_pool(name="outp", bufs=2))

            # phase 3: scores -> P^T (bf16) + column-sum partials
            #   v = (f2bc + f1) * nt       (one fused DVE op; leaky-relu
            #                               commutes with nt >= 0)
            #   w = lrelu(v)               (ScalarE, alpha=0.2)
            #   t = adj*500 + w            (DVE fused, adj read as int32)
            #   P^T = exp(t - 500) via PE-transpose + ScalarE evict; masked
            #   entries give exp(w-500) which underflows to exactly 0.
            for ig in range(NIG):
                s_grp = []
                adjf_grp = []
                for q in range(GRP):
                    ti = ig * GRP + q
                    adj_t = ioa.tile([128, N], I32, tag="adj")
                    nc.gpsimd.dma_start(out=adj_t[:],
                                        in_=adj_d[ti * 128:(ti + 1) * 128, :])
                    nt_t = io.tile([128, N], FP32, tag="nt")
                    nc.scalar.dma_start(out=nt_t[:],
                                        in_=nt_d[ti * 128:(ti + 1) * 128, :])
                    # int32 -> bf16 convert on GpSimd (its only ALU-free op)
                    adjf_t = cvt.tile([128, N], BF16, tag="adjf")
                    nc.gpsimd.tensor_copy(adjf_t[:], adj_t[:])
                    adjf_grp.append(adjf_t)

                    s_t = sc.tile([128, N], FP32, tag="score")
                    nc.vector.scalar_tensor_tensor(
                        out=s_t[:], in0=f2bc[:], scalar=f1_all[:, ti:ti + 1],
                        in1=nt_t[:], op0=mybir.AluOpType.add,
                        op1=mybir.AluOpType.mult)
                    # leaky-relu on DVE (HW ACT Lrelu has a fixed 0.01
                    # slope; alpha is ignored)
                    nc.vector.scalar_tensor_tensor(
                        out=s_t[:], in0=s_t[:], scalar=ALPHA, in1=s_t[:],
                        op0=mybir.AluOpType.mult, op1=mybir.AluOpType.max)
                    s_grp.append(s_t)

                for tj in range(NTI):
                    tp_ps = ps_tp.tile([128, IC], FP32, tag="tp")
                    for q in range(GRP):
                        # quarter = w^T + 500*adj^T (PE applies the mask:
                        # adjf^T @ 500I accumulates onto the transposed
                        # scores; exp bias -500 then zeroes masked entries)
                        quarter = tp_ps[:, q * 128:(q + 1) * 128]
                        nc.tensor.matmul(
                            quarter, s_grp[q][:, tj * 128:(tj + 1) * 128],
                            id128[:], is_transpose=True,
                            start=True, stop=False, skip_group_check=True)
                        nc.tensor.matmul(
                            quarter, adjf_grp[q][:, tj * 128:(tj + 1) * 128],
                            id500[:], start=False, stop=True,
                            skip_group_check=True)
                    nc.scalar.activation(pt_all[:, tj, ig * IC:(ig + 1) * IC],
                                         tp_ps[:],
                                         mybir.ActivationFunctionType.Exp,
                                         bias=neg_shift[:], scale=1.0,
                                         accum_out=cs_parts[:, tj * NIG + ig:
                                                            tj * NIG + ig + 1])
                    if ig == NIG - 1:
                        # colsum finalize + whl2 per tj, so phase-4 matmuls
                        # can start before the whole phase completes.
                        nc.vector.tensor_reduce(
                            cs[:, tj:tj + 1],
                            cs_parts[:].rearrange(
                                "p (t g) -> p t g", g=NIG)[:, tj:tj + 1, :],
                            axis=mybir.AxisListType.X, op=mybir.AluOpType.add)
                        nc.vector.reciprocal(inv_cs[:, tj:tj + 1],
                                             cs[:, tj:tj + 1])
                        nc.vector.tensor_scalar(
                            out=whl2[:, tj * F_out:(tj + 1) * F_out],
                            in0=wh_all[:, tj * F_out:(tj + 1) * F_out],
                            scalar1=level_sb[:, tj:tj + 1],
                            scalar2=inv_cs[:, tj:tj + 1],
                            op0=mybir.AluOpType.mult, op1=mybir.AluOpType.mult)

            # phase 4: h'^T[o, i] = sum_j whl2[j, o] * P^T[j, i], relu on evict
            for ic in range(N // IC):
                mm_ps = ps_mm.tile([F_out, IC], FP32, tag="mm")
                for tj in range(NTI):
                    nc.tensor.matmul(mm_ps[:],
                                     whl2[:, tj * F_out:(tj + 1) * F_out],
                                     pt_all[:, tj, ic * IC:(ic + 1) * IC],
                                     start=(tj == 0), stop=(tj == NTI - 1))
                nc.scalar.activation(hpT[:, ic * IC:(ic + 1) * IC], mm_ps[:],
                                     mybir.ActivationFunctionType.Relu)

            # phase 5: transpose h'^T -> [i, o], DMA out
            for ti in range(NTI):
                ot_ps = ps_tp.tile([128, F_out], FP32, tag="ot")
                nc.tensor.transpose(ot_ps[:], hpT[:, ti * 128:(ti + 1) * 128],
                                    id128[:F_out, :F_out])
                o_sb = out_pool.tile([128, F_out], FP32, tag="osb")
                nc.vector.tensor_copy(o_sb[:], ot_ps[:])
                nc.sync.dma_start(out=out_ap[ti * 128:(ti + 1) * 128, :],
                                  in_=o_sb[:])


def build(cfg: Cfg, repeats: int = 1):
    """Build the single-core Bass program (same program for all cores).

    repeats > 1 emits the full kernel body that many times in one program
    (used only for timing: per-iteration time = diff of wall times).
    """
    nc = bacc.Bacc("TRN2", target_bir_lowering=False, debug=False)
    N, F_in, F_out = cfg.N, cfg.F_in, cfg.F_out
    in_aps = {
        "h": nc.dram_tensor("h", [N, F_in], FP32, kind="ExternalInput").ap(),
        "adj": nc.dram_tensor("adj", [N, N], I32, kind="ExternalInput").ap(),
        "node_type": nc.dram_tensor("node_type", [N, N], FP32,
                                    kind="ExternalInput").ap(),
        "level": nc.dram_tensor("level", [N], FP32, kind="ExternalInput").ap(),
        "W": nc.dram_tensor("W", [F_in, F_out], FP32, kind="ExternalInput").ap(),
        "a": nc.dram_tensor("a", [2 * F_out, 1], FP32, kind="ExternalInput").ap(),
    }
    out_ap = nc.dram_tensor("out", [N, F_out], FP32, kind="ExternalOutput").ap()
    with tile.TileContext(nc) as tc:
        for _ in range(repeats):
            attn_kernel(tc, out_ap, in_aps, cfg)
    nc.compile()
    return nc


_NC_CACHE = {}


def _get_nc(cfg: Cfg):
    key = (cfg.N, cfg.F_in, cfg.F_out)
    if key not in _NC_CACHE:
        _NC_CACHE[key] = build(cfg)
    return _NC_CACHE[key]


def run_on_cores(inputs: dict, cfg: Cfg, trace: bool = False):
    """Shard batch across cores, run, gather. Returns (out[B,N,F_out], bkr)."""
    from concourse.bass_utils import run_bass_kernel_spmd

    B = inputs["h"].shape[0]
    nc = _get_nc(cfg)
    in_maps = []
    for b in range(B):
        in_maps.append({
            "h": np.ascontiguousarray(inputs["h"][b], dtype=np.float32),
            "adj": np.ascontiguousarray(inputs["adj"][b], dtype=np.int32),
            "node_type": np.ascontiguousarray(inputs["node_type"][b],
                                              dtype=np.float32),
            "level": np.ascontiguousarray(inputs["level"][b], dtype=np.float32),
            "W": np.ascontiguousarray(inputs["W"], dtype=np.float32),
            "a": np.ascontiguousarray(inputs["a"], dtype=np.float32),
        })
    bkr = run_bass_kernel_spmd(nc, in_maps, list(range(B)), trace=trace)
    out = np.stack([bkr.results[b]["out"] for b in range(B)], axis=0)
    return out, bkr


def kernel(**inputs) -> np.ndarray:
    cfg = Cfg(N=2048, F_in=256, F_out=64)
    out, _ = run_on_cores(inputs, cfg, trace=False)
    return out.astype(np.float32)


if __name__ == "__main__":
    cfg = Cfg()
    nc = build(cfg)
    print("built ok")
